# revision 58
# baseline (speedup 1.0000x reference)
"""Trainium2 Bass kernel for nn_BiDirectionalFusionModule.

Pure batch data-parallelism: 8 samples -> 8 NeuronCores, each core runs the
full module for one sample.

v2: the two fat 3x3 convs (512ch mask conv, 513ch fusion conv) and the Q
1x1 convs run as fp8(e4m3) DoubleRow matmuls: lhsT [128,2,M] / rhs [128,2,N]
contract 256 channels per instruction at 0.5 cycles/output column (4x bf16
MAC throughput).  The fusion conv output feeds the kernel output directly, so
plain fp8 would cost ~2.7% rel err; instead it runs 3 compensated passes
(W8@x8 + W8@dx8 + dW8@x8, all sharing the s_w*s_x PSUM scale) which restores
~bf16 accuracy at 3/4 of the bf16-conv cost.  Residuals dx8 = Q(s*x - x8) are
exact-scale fp8 (no extra scale needed) computed with one DVE
scalar_tensor_tensor per chunk.  Conv windows use flat pitch-90 rows with a
1-col lead pad so every shifted rhs is a contiguous [128,2,L] slice (wrap
columns land in ignored pitch columns 0/89).

Host-side folds: BN affine -> post-conv scale/bias (divided by s_x*s_w for
fp8 PSUMs); LN affine + softmax scale -> K/V 1x1 conv weights; clip(gamma) ->
post-attention LN affine.  Softmax denominator and max-subtraction cancel
inside the following channel-LayerNorm, so softmax is a bare exp.

Cross-partition (channel) reductions for LN stats via ones-column matmuls;
per-pixel stat vectors are reshaped through DRAM to [121,16] tiles for wide
DVE/ACT math, then DMA-broadcast back.

v3: the two 8x8/stride-8 spatial-reduction convs also run as fp8 DoubleRow
matmuls (weights at scale 1024, rhs = strided [128,2,11,11] windows of the
resident x8 / msk8 fp8 tiles), halving their PE time and weight DMA.  msk8
(= fp8(SX*depth*mask)) is produced by one DVE scalar_tensor_tensor directly
from x bf16 and the broadcast mask, skipping the bf16 intermediate + ACT
pass.  The mask column of the fusion conv is fp8 too (fwm at scale 512,
mask im2col at scale 8, 512*8 = SXW), halving the im2col SBUF footprint.
Phase 3 interleaves the r2d and d2r attention blocks in one loop so the two
directions' serial stats chains (PSUM stats -> DRAM reshape -> rstd math ->
DMA broadcast -> apply) overlap each other, and the fusion conv starts as
soon as the first quantized chunks land.  The mask -> broadcast -> msk8
chain is pipelined in 4 row chunks; phase-2 LN-stat broadcasts use gpsimd
partition_broadcast (no DRAM roundtrip); the apply-phase x8 quantize runs
on ACT to unload the DVE (the busiest engine in the attention region).
560us -> 414us.
"""
import numpy as np
import ml_dtypes
from contextlib import ExitStack

import concourse.bass as bass
from concourse import bacc
import concourse.tile as tile
import concourse.mybir as mybir
from concourse.bass_utils import run_bass_kernel_spmd

F32 = mybir.dt.float32
BF16 = mybir.dt.bfloat16
F8 = mybir.dt.float8e4
AF = mybir.ActivationFunctionType
ALU = mybir.AluOpType
DR = mybir.MatmulPerfMode.DoubleRow
BF = ml_dtypes.bfloat16
E4 = ml_dtypes.float8_e4m3

B, C, H, W = 8, 256, 88, 88
RR = 8
HR = H // RR                # 11
M2 = HR * HR                # 121
N = H * W                   # 7744
PITCH = 90
NF = H * PITCH              # 7920
NFP = NF + 2                # flat fp8 buffer with 1-col lead/tail pad
EPS = 1e-5
CQ = C // 8                 # 32

SX = 16.0                   # activation fp8 scale
SW = 256.0                  # weight fp8 scale
SXW = SX * SW
SSR = 1024.0                # sr-conv weight fp8 scale

BLOCKS = [(i * 5, 5) for i in range(17)] + [(85, 3)]
CHUNK_ROWS = 11             # apply-phase chunking: 8 chunks of 11 rows
N_CHUNKS = H // CHUNK_ROWS
CHUNK_J = CHUNK_ROWS * W // M2   # stats tile free width: 121*J = chunk pixels
OFFS = [(1, 0), (1, 1), (1, 2), (0, 0), (0, 1), (0, 2), (2, 0), (2, 1), (2, 2)]

(CB_S1, CB_T1, CB_SRB0, CB_SRB1, CB_NG0, CB_NB0, CB_NG1, CB_NB1, CB_FS, CB_FT,
 CB_KB0, CB_QB0, CB_KB1, CB_QB1) = range(14)

_CACHE = {}


def _q8(a, s):
    """Quantize to raw e4m3 holding s*a (caller folds 1/s downstream)."""
    a = np.asarray(a, np.float32) * s
    assert np.abs(a).max() < 235.0, f"fp8 overflow risk: {np.abs(a).max()}"
    return a.astype(E4)


def _q8sr(w, s):
    """sr-conv weights [64, 256(in), 256(out)] f32 -> fp8 DR pair layout
    [128, 64*2*256] with in-channel ch = i*128 + p."""
    ws = np.asarray(w, np.float32) * s
    assert np.abs(ws).max() < 235.0, f"fp8 overflow risk: {np.abs(ws).max()}"
    a = ws.astype(E4).reshape(64, 2, 128, 256)   # o, i, p, m
    a = a.transpose(2, 0, 1, 3)                  # p, o, i, m
    return np.ascontiguousarray(a.reshape(128, 64 * 2 * 256))


def _q8pair(w, s):
    """w [9, 512, 256] f32 -> (w8, dw8) fp8 in [256, 9*2*256] pair layout."""
    ws = np.asarray(w, np.float32) * s
    assert np.abs(ws).max() < 235.0
    w8 = ws.astype(E4)
    dw8 = (ws - w8.astype(np.float32)).astype(E4)

    def lay(arr):
        a = arr.reshape(9, 2, 2, 128, 256)       # o, g, i, p, m
        a = a.transpose(1, 3, 0, 2, 4)           # g, p, o, i, m
        return np.ascontiguousarray(a.reshape(256, 9 * 2 * 256))

    return lay(w8), lay(dw8)


def _prep(inputs):
    ii = {k: np.asarray(v, dtype=np.float32) for k, v in inputs.items()}
    scale = float(CQ) ** -0.5

    def fold_bn(g, be, m, v):
        s = g / np.sqrt(v + EPS)
        return s, (0.0 - m) * s + be

    w1T = ii['sm_w1'].transpose(2, 3, 1, 0).reshape(9, 2 * C, C)
    w1q, _ = _q8pair(w1T, SW)
    s1, t1 = fold_bn(ii['sm_g1'], ii['sm_be1'], ii['sm_m1'], ii['sm_v1'])
    t1 = t1 + ii['sm_b1'] * s1
    s1 = s1 / SXW
    w2T = ii['sm_w2'][:, :, 0, 0].T.astype(BF)
    b2 = float(ii['sm_b2'][0])

    fwT = ii['fus_w'][:, :2 * C].transpose(2, 3, 1, 0).reshape(9, 2 * C, C)
    fw8, dfw8 = _q8pair(fwT, SW)
    fwm = _q8(ii['fus_w'][:, 2 * C, :, :].transpose(1, 2, 0).reshape(9, C),
              512.0)
    fs, ft = fold_bn(ii['fus_g'], ii['fus_be'], ii['fus_m'], ii['fus_v'])
    ft = ft + ii['fus_b'] * fs
    fs = fs / SXW

    dirs = {}
    for di, pfx in enumerate(('d2r', 'r2d')):
        g = ii[pfx + '_ln_g']; bl = ii[pfx + '_ln_b']
        kw = ii[pfx + '_k_w'][:, :, 0, 0]; kb = ii[pfx + '_k_b']
        vw = ii[pfx + '_v_w'][:, :, 0, 0]; vb = ii[pfx + '_v_b']
        qw = ii[pfx + '_q_w'][:, :, 0, 0]; qb = ii[pfx + '_q_b']
        gamma = float(np.clip(ii[pfx + '_gamma'], 0.0, 1.0)[0])
        dirs[di] = dict(
            srw8=_q8sr(ii[pfx + '_sr_w'].transpose(2, 3, 1, 0).reshape(64, C, C),
                       SSR),
            srb=ii[pfx + '_sr_b'],
            kwT=(scale * kw * g[None, :]).T.astype(BF),
            kb=scale * (kb + kw @ bl),
            qwT=qw.T, qb=qb,
            vwN=(vw * g[None, :]).T.astype(BF),
            vb=(vb + vw @ bl).astype(BF),
            ng=gamma * ii[pfx + '_norm_g'],
            nb=gamma * ii[pfx + '_norm_b'],
        )

    cb = np.zeros((C, 14), np.float32)
    cb[:, CB_S1] = s1; cb[:, CB_T1] = t1
    cb[:, CB_SRB0] = dirs[0]['srb']; cb[:, CB_SRB1] = dirs[1]['srb']
    cb[:, CB_NG0] = dirs[0]['ng']; cb[:, CB_NB0] = dirs[0]['nb']
    cb[:, CB_NG1] = dirs[1]['ng']; cb[:, CB_NB1] = dirs[1]['nb']
    cb[:, CB_FS] = fs; cb[:, CB_FT] = ft
    cb[:CQ, CB_KB0] = dirs[0]['kb']; cb[:CQ, CB_QB0] = dirs[0]['qb']
    cb[:CQ, CB_KB1] = dirs[1]['kb']; cb[:CQ, CB_QB1] = dirs[1]['qb']
    cbp = np.zeros((128, 28), np.float32)
    cbp[:, 0:14] = cb[0:128]; cbp[:, 14:28] = cb[128:256]

    kq = np.zeros((C, 96), BF)
    kq[:, 0:32] = dirs[0]['kwT']; kq[:, 32:64] = dirs[1]['kwT']
    kq[:, 64:96] = dirs[0]['qwT'].astype(BF)      # d2r Q stays bf16
    # q8w [p, di, i, m] -> [128, 128]  (only di=1 used)
    q8w = np.zeros((128, 2, 2, 32), E4)
    for di in range(2):
        qwT = dirs[di]['qwT']                     # [256, 32]
        q8w[:, di, 0, :] = _q8(qwT[0:128], SW)
        q8w[:, di, 1, :] = _q8(qwT[128:256], SW)
    vw2 = np.concatenate([dirs[0]['vwN'], dirs[1]['vwN']], axis=1)
    vbr = np.concatenate([dirs[0]['vb'], dirs[1]['vb']])[None, :]

    shared = dict(w1q=w1q, w2=w2T, fw8=fw8, dfw8=dfw8, fwm=fwm, cb=cbp, kq=kq,
                  q8w=np.ascontiguousarray(q8w.reshape(128, 128)),
                  vw2=np.ascontiguousarray(vw2), vbr=np.ascontiguousarray(vbr),
                  srw0=dirs[0]['srw8'], srw1=dirs[1]['srw8'])

    rgb = ii['f_rgb']; dep = ii['f_depth']
    in_maps = []
    for i in range(B):
        x32 = np.zeros((2 * C, H, PITCH), np.float32)
        x32[:C, :, 1:89] = rgb[i]
        x32[C:, :, 1:89] = dep[i]
        xf = x32.reshape(2 * C, NF)
        x8 = np.zeros((2 * C, NFP), E4)
        x8[:, 1:NF + 1] = _q8(xf, SX)
        m = dict(shared)
        m['x'] = np.ascontiguousarray(xf.astype(BF))
        m['x8'] = x8
        in_maps.append(m)
    return in_maps, b2


def _dr_conv3x3(nc, psf, units, y0, nr, stop_last):
    """3x3 conv as fp8 DoubleRow matmuls into psf [128, nr*PITCH].

    units: list of (lhsT_fn(o) -> [128,2,128] AP, rhs_tile [128,2,NFP]).
    Flat pitch-90 windows; dy==1 offsets first so the initial matmul covers
    the full region."""
    plan = []
    for dy, dx in OFFS:
        s = dy - 1
        ylo = max(y0, -s); yhi = min(y0 + nr, H - s)
        if ylo >= yhi:
            continue
        for ui in range(len(units)):
            plan.append((dy * 3 + dx, s, ylo, yhi, ui))
    for idx, (o, s, ylo, yhi, ui) in enumerate(plan):
        lf, rt = units[ui]
        out = psf if (ylo == y0 and yhi == y0 + nr) else \
            psf[:, (ylo - y0) * PITCH:(yhi - y0) * PITCH]
        a = (ylo + s) * PITCH + (o % 3)
        nc.tensor.matmul(out, lf(o), rt[:, :, a:a + (yhi - ylo) * PITCH],
                         start=(idx == 0),
                         stop=(stop_last and idx == len(plan) - 1),
                         perf_mode=DR)


def _build(nc, b2, dbg=False, maxphase=4):
    x_d = nc.dram_tensor("x", [2 * C, NF], BF16, kind="ExternalInput")
    x8_d = nc.dram_tensor("x8", [2 * C, NFP], F8, kind="ExternalInput")
    w1q_d = nc.dram_tensor("w1q", [C, 9 * 2 * C], F8, kind="ExternalInput")
    w2_d = nc.dram_tensor("w2", [C, 1], BF16, kind="ExternalInput")
    fw8_d = nc.dram_tensor("fw8", [C, 9 * 2 * C], F8, kind="ExternalInput")
    dfw8_d = nc.dram_tensor("dfw8", [C, 9 * 2 * C], F8, kind="ExternalInput")
    fwm_d = nc.dram_tensor("fwm", [9, C], F8, kind="ExternalInput")
    cb_d = nc.dram_tensor("cb", [128, 28], F32, kind="ExternalInput")
    kq_d = nc.dram_tensor("kq", [C, 96], BF16, kind="ExternalInput")
    q8w_d = nc.dram_tensor("q8w", [128, 128], F8, kind="ExternalInput")
    vw2_d = nc.dram_tensor("vw2", [C, 2 * C], BF16, kind="ExternalInput")
    vbr_d = nc.dram_tensor("vbr", [1, 2 * C], BF16, kind="ExternalInput")
    srw_d = [nc.dram_tensor("srw0", [128, 64 * 2 * C], F8, kind="ExternalInput"),
             nc.dram_tensor("srw1", [128, 64 * 2 * C], F8, kind="ExternalInput")]
    out_d = nc.dram_tensor("out", [C, N], BF16, kind="ExternalOutput")
    dbg_d = {}
    if dbg:
        for nm, shp in [("mask", [1, NF]), ("msk0", [128, N]),
                        ("kvr0", [C, M2]), ("kvr1", [C, M2]),
                        ("kvn0", [C, M2]), ("kvn1", [C, M2]),
                        ("k0", [32, M2]), ("k1", [32, M2]),
                        ("v0", [M2, C]), ("v1", [M2, C]),
                        ("h1", [C, N]), ("q1", [32, N]),
                        ("num0", [C, N]), ("num1", [C, N]),
                        ("rm0", [2, N]), ("rm1", [2, N]),
                        ("st0", [2, N]), ("st1", [2, N]),
                        ("enh0", [128, NF]), ("enh2", [128, NF])]:
            dt_ = F32 if nm.startswith("st") else BF16
            dbg_d[nm] = nc.dram_tensor("dbg_" + nm, shp, dt_, kind="ExternalOutput")
        for nm in ("x8e0", "x8e1", "dx8e0", "dx8e1"):
            dbg_d[nm] = nc.dram_tensor("dbg_" + nm, [2 * 128, NFP], F8,
                                       kind="ExternalOutput")

    with tile.TileContext(nc) as tc:
        es = ExitStack()
        with es, tc.tile_pool(name="dram", bufs=1, space="DRAM") as dpool:
            gp = es.enter_context(tc.tile_pool(name="gp", bufs=1))

            cb_sb = gp.tile([128, 28], F32, name="cb_sb")

            def cbc(col, half):
                return cb_sb[:, half * 14 + col:half * 14 + col + 1]

            kq_sb = gp.tile([128, 2, 96], BF16, name="kq_sb")
            q8w_sb = gp.tile([128, 2, 2, 32], F8, name="q8w_sb")
            w2_sb = gp.tile([128, 2, 1], BF16, name="w2_sb")
            ones_bf = gp.tile([128, 1], BF16, name="ones_bf")
            nc.vector.memset(ones_bf, 1.0)
            zrow = gp.tile([1, PITCH], BF16, name="zrow")
            nc.vector.memset(zrow, 0.0)
            zrow8 = gp.tile([1, PITCH], F8, name="zrow8")
            nc.vector.memset(zrow8, 0.0)
            eps_sb = gp.tile([128, 1], F32, name="eps_sb")
            nc.vector.memset(eps_sb, EPS)
            b2_sb = gp.tile([128, 1], F32, name="b2_sb")
            nc.vector.memset(b2_sb, b2)

            mask_dram = dpool.tile([1, PITCH * PITCH], BF16, name="mask_dram")
            mask8_dram = dpool.tile([1, PITCH * PITCH], F8, name="mask8_dram")
            pool_x = es.enter_context(tc.tile_pool(name="px", bufs=1))

            with tc.tile_pool(name="srp", bufs=6) as srp:
              px8r_es = ExitStack()
              px8r = px8r_es.enter_context(tc.tile_pool(name="px8r", bufs=1))
              # ============== Phase 1: conv1 + spatial mask ==============
              with tc.tile_pool(name="pms", bufs=1) as pms:
                mask_sb = pms.tile([1, H, PITCH], BF16, name="mask_sb")
                nc.gpsimd.memset(mask_sb, 0.0)
                mask3 = mask_sb  # [1, 88, 90]
                with tc.tile_pool(name="pw1", bufs=1) as pw1, \
                     tc.tile_pool(name="px8", bufs=1) as px8, \
                     tc.tile_pool(name="ps1", bufs=3, space="PSUM") as ps1, \
                     tc.tile_pool(name="ps1m", bufs=2, space="PSUM") as ps1m, \
                     tc.tile_pool(name="ev1", bufs=2) as ev:
                    if maxphase < 1:
                        return
                    # conv inputs first: the first conv matmul gates on these
                    # rgb group outlives phase 1 (r2d sr-conv rhs in phase 2)
                    x8_sb = [px8r.tile([128, 2, NFP], F8, name="x8_0"),
                             px8.tile([128, 2, NFP], F8, name="x8_1")]
                    x8v = x8_d.rearrange("(c p) f -> c p f", p=128)
                    w1qv = w1q_d.rearrange(
                        "(g p) (o i m) -> g p o i m", p=128, o=9, i=2)
                    w1q_sb = [pw1.tile([128, 9, 2, C], F8, name=f"w1q_{g}",
                                       tag=f"w1q_{g}") for g in range(2)]
                    nc.sync.dma_start(out=w1q_sb[0], in_=w1qv[0])
                    for rc in range(4):
                        a0 = 1 + rc * 22 * PITCH
                        a1 = 1 + (rc + 1) * 22 * PITCH
                        if rc == 0:
                            a0 = 0
                        if rc == 3:
                            a1 = NFP
                        for g in range(2):
                            for i in range(2):
                                nc.sync.dma_start(out=x8_sb[g][:, i, a0:a1],
                                                  in_=x8v[2 * g + i][:, a0:a1])
                        if rc == 0:
                            nc.sync.dma_start(out=w1q_sb[1], in_=w1qv[1])
                    nc.sync.dma_start(out=cb_sb, in_=cb_d[:, :])
                    for t in range(2):
                        nc.sync.dma_start(out=w2_sb[:, t, :],
                                          in_=w2_d.rearrange("(t p) q -> t p q", p=128)[t])
                    for t in range(2):
                        nc.sync.dma_start(out=kq_sb[:, t, :],
                                          in_=kq_d.rearrange("(t p) q -> t p q", p=128)[t])
                    nc.sync.dma_start(
                        out=q8w_sb,
                        in_=q8w_d.rearrange("p (d i m) -> p d i m", d=2, i=2))
                    # x bf16 (8MB) is not needed until the mask multiply and
                    # the apply phase: load it after the conv-critical tensors
                    x_sb = [pool_x.tile([128, H, PITCH], BF16, name=f"x{t}",
                                        tag=f"x{t}") for t in range(4)]
                    xv = x_d.rearrange("(t p) (h q) -> t p h q", p=128, q=PITCH)
                    for rc in range(4):
                        rs = slice(rc * 22, (rc + 1) * 22)
                        for t in range(4):
                            nc.sync.dma_start(out=x_sb[t][:, rs, :],
                                              in_=xv[t][:, rs, :])

                    for y0, nr in BLOCKS:
                        h1b = []
                        for cb_i in range(2):
                            ps = ps1.tile([128, nr, PITCH], F32, name="c1ps",
                                          tag="c1ps")
                            psf = ps.rearrange("p r w -> p (r w)")
                            units = [
                                (lambda o, g=g, cb_i=cb_i:
                                     w1q_sb[g][:, o, :, cb_i * 128:(cb_i + 1) * 128],
                                 x8_sb[g]) for g in range(2)]
                            _dr_conv3x3(nc, psf, units, y0, nr, stop_last=True)
                            h1t = ev.tile([128, nr, W], BF16, name="h1t",
                                          tag=f"h1t{cb_i}")
                            nc.scalar.activation(h1t, ps[:, :, 1:89], AF.Relu,
                                                 bias=cbc(CB_T1, cb_i),
                                                 scale=cbc(CB_S1, cb_i))
                            h1b.append(h1t)
                            if dbg and cb_i == 0:
                                nc.sync.dma_start(
                                    out=dbg_d["h1"][0:128,
                                                    y0 * W:(y0 + nr) * W],
                                    in_=h1t.rearrange("p r w -> p (r w)"))
                        mps = ps1m.tile([1, nr * W], F32, name="mps", tag="mps")
                        for cb_i in range(2):
                            nc.tensor.matmul(
                                mps, w2_sb[:, cb_i, :],
                                h1b[cb_i].rearrange("p r w -> p (r w)"),
                                start=(cb_i == 0), stop=(cb_i == 1))
                        nc.scalar.activation(mask3[:, y0:y0 + nr, 1:89], mps,
                                             AF.Sigmoid, bias=b2_sb[0:1, :], scale=1.0)
                # prefetch first r2d srw chunks ahead of the mask-gated DMAs
                preload = {}
                for grp in (0, 1, 2):
                    wp = srp.tile([128, 16, 2, C], F8, name="wch", tag="wch")
                    nc.sync.dma_start(out=wp, in_=srw_d[1].rearrange(
                        "p (g o i m) -> g p o i m", g=4, o=16, i=2)[grp])
                    preload[grp] = wp
                # mask -> zero-padded 90x90 in DRAM, in 4 row chunks so the
                # broadcast + msk8 quantize pipeline behind the sigmoid blocks
                nc.sync.dma_start(out=mask_dram[:, 0:PITCH], in_=zrow)
                nc.sync.dma_start(out=mask_dram[:, 89 * PITCH:], in_=zrow)
                mflat = mask_sb.rearrange("o h q -> o (h q)")
                for rc in range(4):
                    nc.sync.dma_start(
                        out=mask_dram[:, (1 + 22 * rc) * PITCH:
                                      (1 + 22 * rc + 22) * PITCH],
                        in_=mflat[:, 22 * rc * PITCH:(22 * rc + 22) * PITCH])
                # fp8 copy at scale 8 for the fusion-conv mask column
                # (fwm is fp8 at scale 512; 512*8 = SXW keeps the PSUM scale)
                m8_stage = pms.tile([1, H, PITCH], F8, name="m8_stage")
                nc.scalar.activation(m8_stage, mask_sb, AF.Identity, scale=8.0)
                nc.sync.dma_start(out=mask8_dram[:, 0:PITCH], in_=zrow8)
                nc.sync.dma_start(out=mask8_dram[:, 89 * PITCH:], in_=zrow8)
                nc.sync.dma_start(out=mask8_dram[:, PITCH:89 * PITCH],
                                  in_=m8_stage.rearrange("o h q -> o (h q)"))
                if dbg:
                    nc.sync.dma_start(out=dbg_d["mask"][:, :],
                                      in_=mask_sb.rearrange("o h q -> o (h q)"))
              if maxphase < 2:
                  return
              pmsk_es = ExitStack()
              pmsk = pmsk_es.enter_context(tc.tile_pool(name="pmsk", bufs=1))
              msk8_sb = gp.tile([128, 2, N], F8, name="msk8")
              p2c_es = ExitStack()
              p2c = p2c_es.enter_context(tc.tile_pool(name="p2c", bufs=1))
              vw2_sb = p2c.tile([128, 2, 2 * C], BF16, name="vw2_sb")
              vbr_sb = p2c.tile([1, 2 * C], BF16, name="vbr_sb")
              ones1_bf = p2c.tile([1, M2], BF16, name="ones1_bf")
              nc.vector.memset(ones1_bf, 1.0)
              for t in range(2):
                  nc.sync.dma_start(out=vw2_sb[:, t, :],
                                    in_=vw2_d.rearrange("(t p) q -> t p q", p=128)[t])
              nc.sync.dma_start(out=vbr_sb, in_=vbr_d[:, :])
              mask_b = pmsk.tile([128, H, W], BF16, name="mask_b")
              m90 = mask_dram.rearrange("o (h q) -> o h q", q=PITCH)
              m8v = [msk8_sb[:, t, :].rearrange("p (a b) -> p a b", b=W)
                     for t in range(2)]
              for rc in range(4):
                  rs = slice(rc * 22, (rc + 1) * 22)
                  nc.sync.dma_start(
                      out=mask_b[:, rs, :],
                      in_=m90[:, 1 + 22 * rc:1 + 22 * rc + 22, 1:89]
                      .to_broadcast([128, 22, W]))
                  for t in range(2):
                      nc.vector.scalar_tensor_tensor(
                          out=m8v[t][:, rs, :],
                          in0=x_sb[2 + t][:, rs, 1:89], scalar=SX,
                          in1=mask_b[:, rs, :],
                          op0=ALU.mult, op1=ALU.mult)
              if dbg:
                  mt = pmsk.tile([128, H, W], BF16, name="msk0d")
                  nc.vector.tensor_tensor(out=mt, in0=x_sb[2][:, :, 1:89],
                                          in1=mask_b, op=ALU.mult)
                  nc.sync.dma_start(out=dbg_d["msk0"][:, :],
                                    in_=mt.rearrange("p a b -> p (a b)"))

              # ====== Phase 2: sr-conv + channel-LN + K / V^T (r2d then d2r) ======
              kvs = {}
              with tc.tile_pool(name="ps2", bufs=1, space="PSUM") as ps2, \
                   tc.tile_pool(name="ps2s", bufs=1, space="PSUM") as ps2s, \
                   tc.tile_pool(name="ev2", bufs=2) as ev:
                  for di in (1, 0):
                      if di == 0:
                          kv8 = msk8_sb.rearrange("p i (h w) -> p i h w", w=W)
                      else:
                          kv8 = x8_sb[0][:, :, 1:1 + H * PITCH].rearrange(
                              "p i (h q) -> p i h q", q=PITCH)
                      srps = [ps2.tile([128, M2], F32, name="srps", tag=f"srps{i}")
                              for i in range(2)]
                      for grp in range(4):
                          if di == 1 and grp in preload:
                              wch = preload[grp]
                          else:
                              wch = srp.tile([128, 16, 2, C], F8, name="wch",
                                             tag="wch")
                              nc.sync.dma_start(out=wch, in_=srw_d[di].rearrange(
                                  "p (g o i m) -> g p o i m", g=4, o=16, i=2)[grp])
                          for o in range(16):
                              off = grp * 16 + o
                              dy, dx = off // 8, off % 8
                              # di=1 reads the pitch-90 x8 layout where image
                              # col j sits at view col 1+j
                              x0 = dx + 1 if di == 1 else dx
                              rhs = kv8[:, :, dy::RR, x0:x0 + 81:RR]
                              for cb_i in range(2):
                                  nc.tensor.matmul(
                                      srps[cb_i],
                                      wch[:, o, :, cb_i * 128:(cb_i + 1) * 128],
                                      rhs, start=(off == 0), stop=(off == 63),
                                      perf_mode=DR)
                      kvr = []
                      for cb_i in range(2):
                          kt = ev.tile([128, M2], BF16, name="kvr", tag=f"kvr{cb_i}")
                          nc.scalar.activation(kt, srps[cb_i], AF.Identity,
                                               bias=cbc(CB_SRB0 + di, cb_i),
                                               scale=1.0 / (SX * SSR))
                          kvr.append(kt)
                          if dbg:
                              nc.sync.dma_start(
                                  out=dbg_d[f"kvr{di}"][cb_i * 128:(cb_i + 1) * 128, :],
                                  in_=kt)
                      mu_ps = ps2s.tile([1, M2], F32, name="mups", tag="mups")
                      sq_ps = ps2s.tile([1, M2], F32, name="sqps", tag="sqps")
                      for cb_i in range(2):
                          sq = ev.tile([128, M2], BF16, name="sqkv", tag="sqkv")
                          nc.vector.tensor_tensor(out=sq, in0=kvr[cb_i], in1=kvr[cb_i],
                                                  op=ALU.mult)
                          nc.tensor.matmul(mu_ps, ones_bf, kvr[cb_i],
                                           start=(cb_i == 0), stop=(cb_i == 1))
                          nc.tensor.matmul(sq_ps, ones_bf, sq,
                                           start=(cb_i == 0), stop=(cb_i == 1))
                      mu = ev.tile([1, M2], F32, name="mukv", tag="mukv")
                      nc.vector.tensor_scalar(mu, mu_ps, 1.0 / C, None, ALU.mult)
                      ms = ev.tile([1, M2], F32, name="mskv", tag="mskv")
                      nc.vector.tensor_scalar(ms, sq_ps, 1.0 / C, None, ALU.mult)
                      mu2 = ev.tile([1, M2], F32, name="mu2kv", tag="mu2kv")
                      nc.vector.tensor_tensor(out=mu2, in0=mu, in1=mu, op=ALU.mult)
                      nc.vector.tensor_tensor(out=ms, in0=ms, in1=mu2, op=ALU.subtract)
                      sd = ev.tile([1, M2], F32, name="sdkv", tag="sdkv")
                      nc.scalar.activation(sd, ms, AF.Sqrt, bias=eps_sb[0:1, :],
                                           scale=1.0)
                      rstd = ev.tile([1, M2], F32, name="rstdkv", tag="rstdkv")
                      nc.vector.reciprocal(rstd, sd)
                      nrm_bf = ev.tile([1, 2, M2], BF16, name="nrmbf", tag="nrmbf")
                      nc.vector.tensor_copy(nrm_bf[:, 0, :], rstd)
                      murm = ev.tile([1, M2], F32, name="murm", tag="murm")
                      nc.vector.tensor_tensor(out=murm, in0=mu, in1=rstd, op=ALU.mult)
                      nc.vector.tensor_copy(nrm_bf[:, 1, :], murm)
                      rstd_b = ev.tile([128, M2], BF16, name="rstdb", tag="rstdb")
                      nc.gpsimd.partition_broadcast(rstd_b, nrm_bf[:, 0, :])
                      mur_b = ev.tile([128, M2], BF16, name="murb", tag="murb")
                      nc.gpsimd.partition_broadcast(mur_b, nrm_bf[:, 1, :])
                      kvn = []
                      for cb_i in range(2):
                          kn = p2c.tile([128, M2], BF16, name=f"kvn{di}{cb_i}")
                          nc.vector.tensor_tensor(out=kn, in0=kvr[cb_i], in1=rstd_b,
                                                  op=ALU.mult)
                          nc.vector.tensor_tensor(out=kn, in0=kn, in1=mur_b,
                                                  op=ALU.subtract)
                          kvn.append(kn)
                          if dbg:
                              nc.sync.dma_start(
                                  out=dbg_d[f"kvn{di}"][cb_i * 128:(cb_i + 1) * 128, :],
                                  in_=kn)
                      kps = ps2s.tile([32, M2], F32, name="kps", tag="kps")
                      for cb_i in range(2):
                          nc.tensor.matmul(kps, kq_sb[:, cb_i, di * 32:di * 32 + 32],
                                           kvn[cb_i], start=(cb_i == 0),
                                           stop=(cb_i == 1))
                      k_bf = gp.tile([32, M2], BF16, name=f"k_bf{di}")
                      nc.scalar.activation(
                          k_bf, kps, AF.Identity,
                          bias=cb_sb[0:32, CB_KB0 + 2 * di:CB_KB0 + 2 * di + 1],
                          scale=1.0)
                      vps = ps2.tile([M2, C], F32, name="vps", tag="vps")
                      for cb_i in range(2):
                          nc.tensor.matmul(vps, kvn[cb_i],
                                           vw2_sb[:, cb_i, di * C:(di + 1) * C],
                                           start=(cb_i == 0), stop=False)
                      nc.tensor.matmul(vps, ones1_bf, vbr_sb[:, di * C:(di + 1) * C],
                                       start=False, stop=True)
                      v_bf = gp.tile([M2, C], BF16, name=f"v_bf{di}")
                      vcol = ev.tile([M2, 1], F32, name="vcol", tag="vcol")
                      nc.scalar.activation(v_bf, vps, AF.Identity, accum_out=vcol)
                      vc_bf = gp.tile([M2, 1], BF16, name=f"vc_bf{di}")
                      nc.vector.tensor_scalar(vc_bf, vcol, 1.0 / C, None, ALU.mult)
                      if dbg:
                          nc.sync.dma_start(out=dbg_d[f"k{di}"][:, :], in_=k_bf)
                          nc.sync.dma_start(out=dbg_d[f"v{di}"][:, :], in_=v_bf)
                      kvs[di] = (k_bf, v_bf, vc_bf)
              p2c_es.close()
              pmsk_es.close()
              px8r_es.close()

            # ====== Phase 3: attention + LN + residual (r2d then d2r) ======
            # Software-pipelined: per-block LN stats stream into per-chunk
            # apply + fp8 quantize through an 11-row numerator ring, so the
            # DVE/ACT apply work overlaps the PE attention blocks.  Phase-4
            # pools are opened here so the fusion conv can start as soon as
            # quantized chunks land (no pool-release serialization).
            if maxphase < 3:
                return
            pq_es = ExitStack()
            pq = pq_es.enter_context(tc.tile_pool(name="pq", bufs=1))
            x8e_sb = [pq.tile([128, 2, NFP], F8, name=f"x8e{g}") for g in range(2)]
            dx8e_sb = [pq.tile([128, 2, NFP], F8, name=f"dx8e{g}") for g in range(2)]
            for g in range(2):
                for i in range(2):
                    nc.gpsimd.memset(x8e_sb[g][:, i, 0:1], 0.0)
                    nc.gpsimd.memset(x8e_sb[g][:, i, NF + 1:NF + 2], 0.0)
                    nc.gpsimd.memset(dx8e_sb[g][:, i, 0:1], 0.0)
                    nc.gpsimd.memset(dx8e_sb[g][:, i, NF + 1:NF + 2], 0.0)
            CR = CHUNK_ROWS
            with tc.tile_pool(name="ps3", bufs=1, space="PSUM") as ps3, \
                 tc.tile_pool(name="ps3n", bufs=1, space="PSUM") as ps3n, \
                 tc.tile_pool(name="ps4", bufs=2, space="PSUM") as ps4, \
                 tc.tile_pool(name="ev3", bufs=2) as ev, \
                 tc.tile_pool(name="nump", bufs=1) as num_p, \
                 tc.tile_pool(name="rbp", bufs=1) as rb_p, \
                 tc.tile_pool(name="pfw", bufs=1) as pfw, \
                 tc.tile_pool(name="pim2", bufs=1) as pim2, \
                 tc.tile_pool(name="ev4", bufs=2) as ev4:
              fw8_sb = [pfw.tile([128, 9, 2, C], F8, name=f"fw8_{g}")
                        for g in range(2)]
              dfw8_sb = [pfw.tile([128, 9, 2, C], F8, name=f"dfw8_{g}")
                         for g in range(2)]
              fwm_sb = pfw.tile([9, C], F8, name="fwm_sb")
              im2b = pim2.tile([9, PITCH * PITCH + 4], F8, name="im2b")
              # per-direction state: both directions' attention blocks are
              # interleaved below so their serial stats chains overlap
              dstate = {}
              for di in (1, 0):
                  stats_dram = dpool.tile([2, N], F32, name=f"stats_dram{di}",
                                          tag="stats_dram", bufs=2)
                  rmur_dram = dpool.tile([2, N], BF16, name=f"rmur_dram{di}",
                                         tag="rmur_dram", bufs=2)
                  # numerator ring: 2 chunk slots of CR rows per cb half
                  num_sb = [num_p.tile([128, 2, CR, W], BF16,
                                       name=f"num{di}{cb_i}", tag=f"num{di}{cb_i}")
                            for cb_i in range(2)]
                  dstate[di] = (stats_dram, rmur_dram, num_sb)

              def emit_apply(di, ch):
                  stats_dram, rmur_dram, num_sb = dstate[di]
                  if True:
                      c0 = ch * CR * W
                      cn = CR * W
                      st = ev.tile([121, 2, CHUNK_J], F32, name="st", tag="st")
                      nc.sync.dma_start(
                          out=st, in_=stats_dram[:, c0:c0 + cn]
                          .rearrange("t (p j) -> p t j", j=CHUNK_J))
                      mu2_t = ev.tile([121, CHUNK_J], F32, name="mu2_t", tag="mu2_t")
                      nc.vector.tensor_tensor(out=mu2_t, in0=st[:, 0, :],
                                              in1=st[:, 0, :], op=ALU.mult)
                      ms_t = ev.tile([121, CHUNK_J], F32, name="ms_t", tag="ms_t")
                      nc.vector.tensor_tensor(out=ms_t, in0=st[:, 1, :], in1=mu2_t,
                                              op=ALU.subtract)
                      sd_t = ev.tile([121, CHUNK_J], F32, name="sd_t", tag="sd_t")
                      nc.scalar.activation(sd_t, ms_t, AF.Sqrt,
                                           bias=eps_sb[0:121, :], scale=1.0)
                      r_t = ev.tile([121, CHUNK_J], F32, name="r_t", tag="r_t")
                      nc.vector.reciprocal(r_t, sd_t)
                      rm_bf = ev.tile([121, 2, CHUNK_J], BF16, name="rm_bf",
                                      tag="rm_bf")
                      nc.vector.tensor_copy(rm_bf[:, 0, :], r_t)
                      nc.vector.tensor_tensor(out=mu2_t, in0=st[:, 0, :], in1=r_t,
                                              op=ALU.mult)
                      nc.vector.tensor_copy(rm_bf[:, 1, :], mu2_t)
                      nc.sync.dma_start(
                          out=rmur_dram[:, c0:c0 + cn]
                          .rearrange("t (p j) -> p t j", j=CHUNK_J), in_=rm_bf)
                      rb = rb_p.tile([128, 2, CR, W], BF16, name="rb", tag="rb")
                      for ti in range(2):
                          nc.sync.dma_start(
                              out=rb[:, ti],
                              in_=rmur_dram[ti:ti + 1, c0:c0 + cn]
                              .to_broadcast([128, cn])
                              .rearrange("p (r w) -> p r w", w=W))
                      rows = slice(ch * CR, (ch + 1) * CR)
                      for cb_i in range(2):
                          seg = num_sb[cb_i][:, ch % 2]
                          nc.vector.tensor_tensor(out=seg, in0=seg, in1=rb[:, 0],
                                                  op=ALU.mult)
                          nc.vector.tensor_tensor(out=seg, in0=seg, in1=rb[:, 1],
                                                  op=ALU.subtract)
                          nc.vector.tensor_scalar(seg, seg,
                                                  cbc(CB_NG0 + 2 * di, cb_i),
                                                  cbc(CB_NB0 + 2 * di, cb_i),
                                                  ALU.mult, ALU.add)
                          t = 2 * di + cb_i
                          nc.vector.tensor_tensor(
                              out=x_sb[t][:, rows, 1:89], in0=seg,
                              in1=x_sb[t][:, rows, 1:89], op=ALU.add)
                          # quantize the enhanced chunk for the fp8 fusion conv
                          g, i = t // 2, t % 2
                          fl0 = ch * CR * PITCH
                          fln = CR * PITCH
                          xflat = x_sb[t].rearrange("p h q -> p (h q)")[
                              :, fl0:fl0 + fln]
                          x8v_ = x8e_sb[g][:, i, 1 + fl0:1 + fl0 + fln]
                          nc.scalar.activation(x8v_, xflat, AF.Identity, scale=SX)
                          up = ev.tile([128, CR * PITCH], BF16,
                                       name="up8", tag="up8")
                          nc.scalar.activation(up, x8v_, AF.Identity)
                          nc.vector.scalar_tensor_tensor(
                              out=dx8e_sb[g][:, i, 1 + fl0:1 + fl0 + fln],
                              in0=xflat, scalar=SX, in1=up,
                              op0=ALU.mult, op1=ALU.subtract)

              # stage the fusion-conv weights + mask im2col before attention
              fw8v = fw8_d.rearrange(
                  "(g p) (o i m) -> g p o i m", p=128, o=9, i=2)
              dfw8v = dfw8_d.rearrange(
                  "(g p) (o i m) -> g p o i m", p=128, o=9, i=2)
              for g in range(2):
                  nc.sync.dma_start(out=fw8_sb[g], in_=fw8v[g])
              for g in range(2):
                  nc.sync.dma_start(out=dfw8_sb[g], in_=dfw8v[g])
              nc.sync.dma_start(out=fwm_sb, in_=fwm_d[:, :])
              nc.gpsimd.memset(im2b, 0.0)
              for dy in range(3):
                  for dx in range(3):
                      j = dy * 3 + dx
                      joff = dy * PITCH + dx
                      if joff == 0:
                          nc.sync.dma_start(
                              out=im2b[0:1, 1:1 + PITCH * PITCH],
                              in_=mask8_dram[:, :])
                      else:
                          nc.sync.dma_start(
                              out=im2b[j:j + 1, 0:PITCH * PITCH - joff + 1],
                              in_=mask8_dram[:, joff - 1:])

              def emit_fusion(y0, nr):
                  for cb_i in range(2):
                      ps = ps4.tile([128, nr, PITCH], F32, name="c2ps", tag="c2ps")
                      psf = ps.rearrange("p r w -> p (r w)")
                      units = []
                      for g in (1, 0):
                          units.append(
                              (lambda o, g=g, cb_i=cb_i:
                                   fw8_sb[g][:, o, :, cb_i * 128:(cb_i + 1) * 128],
                               x8e_sb[g]))
                          units.append(
                              (lambda o, g=g, cb_i=cb_i:
                                   fw8_sb[g][:, o, :, cb_i * 128:(cb_i + 1) * 128],
                               dx8e_sb[g]))
                          units.append(
                              (lambda o, g=g, cb_i=cb_i:
                                   dfw8_sb[g][:, o, :, cb_i * 128:(cb_i + 1) * 128],
                               x8e_sb[g]))
                      _dr_conv3x3(nc, psf, units, y0, nr, stop_last=False)
                      nc.tensor.matmul(
                          psf, fwm_sb[:, cb_i * 128:(cb_i + 1) * 128],
                          im2b[:, y0 * PITCH:(y0 + nr) * PITCH],
                          start=False, stop=True)
                      o_t = ev4.tile([128, nr, W], BF16, name="o_t", tag="o_t")
                      nc.scalar.activation(o_t, ps[:, :, 1:89], AF.Relu,
                                           bias=cbc(CB_FT, cb_i),
                                           scale=cbc(CB_FS, cb_i))
                      nc.sync.dma_start(
                          out=out_d[cb_i * 128:(cb_i + 1) * 128,
                                    y0 * W:(y0 + nr) * W],
                          in_=o_t.rearrange("p r w -> p (r w)"))

              TRIG = {2: 0, 4: 1, 6: 2, 8: 3, 10: 4, 13: 5, 15: 6, 17: 7}
              # fusion blocks runnable once chunks <= ch are quantized:
              # block rows y0-1 .. y0+nr must lie within rows < 11*(ch+1)
              FTRIG = {0: [0, 5], 1: [10, 15], 2: [20, 25], 3: [30, 35],
                       4: [40, 45], 5: [50, 55, 60], 6: [65, 70],
                       7: [75, 80, 85]}
              for bi, (y0, nr) in enumerate(BLOCKS):
                  for di in (1, 0):
                      stats_dram, rmur_dram, num_sb = dstate[di]
                      k_bf, v_bf, vc_bf = kvs[di]
                      nn = nr * W
                      qps = ps3n.tile([32, nr, W], F32, name="qps", tag="qps")
                      if di == 0:
                          for ci in range(2):
                              nc.tensor.matmul(
                                  qps.rearrange("p r w -> p (r w)"),
                                  kq_sb[:, ci, 64:96],
                                  x_sb[ci][:, y0:y0 + nr, 1:89],
                                  start=(ci == 0), stop=(ci == 1))
                      else:
                          nc.tensor.matmul(
                              qps.rearrange("p r w -> p (r w)"),
                              q8w_sb[:, di, :, :],
                              msk8_sb[:, :, y0 * W:(y0 + nr) * W],
                              start=True, stop=True, perf_mode=DR)
                      q_bf = ev.tile([32, nr, W], BF16, name="q_bf", tag="q_bf")
                      nc.scalar.activation(
                          q_bf, qps, AF.Identity,
                          bias=cb_sb[0:32, CB_QB0 + 2 * di:CB_QB0 + 2 * di + 1],
                          scale=(1.0 if di == 0 else 1.0 / SXW))
                      sps = ps3.tile([M2, nn], F32, name="sps", tag="sps")
                      nc.tensor.matmul(sps, k_bf,
                                       q_bf.rearrange("p r w -> p (r w)"),
                                       start=True, stop=True)
                      e_bf = ev.tile([M2, nn], BF16, name="e_bf", tag="e_bf")
                      nc.scalar.activation(e_bf, sps, AF.Exp)
                      mu_ps = ps3n.tile([1, nn], F32, name="amups", tag="astps",
                                        bufs=2)
                      nc.tensor.matmul(mu_ps, vc_bf, e_bf, start=True, stop=True)
                      sq_ps = ps3n.tile([1, nn], F32, name="asqps", tag="astps",
                                        bufs=2)
                      for cb_i in range(2):
                          nps = ps3.tile([128, nn], F32, name="nps",
                                         tag=f"nps{cb_i}", bufs=1)
                          nc.tensor.matmul(nps,
                                           v_bf[:, cb_i * 128:(cb_i + 1) * 128],
                                           e_bf, start=True, stop=True)
                          npsv = nps.rearrange("p (r w) -> p r w", w=W)
                          r = y0
                          while r < y0 + nr:
                              k = r // CR
                              r2 = min((k + 1) * CR, y0 + nr)
                              nc.vector.tensor_copy(
                                  num_sb[cb_i][:, k % 2, r - k * CR:r2 - k * CR, :],
                                  npsv[:, r - y0:r2 - y0, :])
                              r = r2
                          nsq = ev.tile([128, nn], BF16, name="nsq", tag="nsq")
                          nc.scalar.activation(nsq, nps, AF.Square, scale=0.0625)
                          nc.tensor.matmul(sq_ps, ones_bf, nsq,
                                           start=(cb_i == 0), stop=(cb_i == 1))
                      str2 = ev.tile([1, 2, nn], F32, name="str2", tag="str2")
                      nc.vector.tensor_copy(str2[:, 0, :], mu_ps)
                      nc.vector.tensor_copy(str2[:, 1, :], sq_ps)
                      for ti in range(2):
                          nc.sync.dma_start(
                              out=stats_dram[ti:ti + 1, y0 * W:y0 * W + nn],
                              in_=str2[:, ti, :])
                      if bi in TRIG:
                          emit_apply(di, TRIG[bi])
              if dbg:
                  for di in (1, 0):
                      nc.sync.dma_start(out=dbg_d[f"st{di}"][:, :],
                                        in_=dstate[di][0][:, :])
                      nc.sync.dma_start(out=dbg_d[f"rm{di}"][:, :],
                                        in_=dstate[di][1][:, :])

              if dbg:
                  for t in (0, 2):
                      nc.sync.dma_start(
                          out=dbg_d[f"enh{t}"][:, :],
                          in_=x_sb[t].rearrange("p h q -> p (h q)"))
                  for g in range(2):
                      nc.sync.dma_start(
                          out=dbg_d[f"x8e{g}"][:, :].rearrange(
                              "(i p) f -> p i f", p=128),
                          in_=x8e_sb[g])
                      nc.sync.dma_start(
                          out=dbg_d[f"dx8e{g}"][:, :].rearrange(
                              "(i p) f -> p i f", p=128),
                          in_=dx8e_sb[g])

              # ============== Phase 4: fusion conv (pools pre-opened) =========
              if maxphase >= 4:
                  for y0, nr in BLOCKS:
                      emit_fusion(y0, nr)
            pq_es.close()
    nc.finalize()
    return nc


def kernel(**inputs):
    in_maps, b2 = _prep(inputs)
    key = ("nc", round(b2, 9))
    if key not in _CACHE:
        nc = bacc.Bacc("TRN2", target_bir_lowering=False, debug=False)
        _build(nc, b2)
        _CACHE[key] = nc
    nc = _CACHE[key]
    res = run_bass_kernel_spmd(nc, in_maps, list(range(B)))
    return np.stack([np.asarray(res.results[i]["out"], np.float32).reshape(C, H, W)
                     for i in range(B)])

